# revision 58
# baseline (speedup 1.0000x reference)
"""AttentionBlock (GroupNorm + single-head spatial attention + residual) on 8
NeuronCores — fp8 DoubleRow edition.

Data-parallel over batch: 16 batch elements -> 2 per core, software-pipelined.

All five big matmuls (qkv, v, scores, attn@V, out-proj) run as fp8e4m3
DoubleRow matmuls (2 fp8 weights per PE cell -> K=256 per instruction, 2x
the bf16/fp32r row rate). The scaling scheme keeps every fp8 operand in the
healthy e4m3 range and folds all compensation into existing free scale
slots (measured end-to-end rel err ~1.4e-2 vs the 2e-2 gate):
  - weights scaled x16 on host (their natural sigma ~1/sqrt(C)=0.044 sits in
    the fp8 denormal range; x16 moves it to ~0.7)
  - q8,k8 = 16*(q,k); scores psum = 256*S -> exp scale folds 1/256
  - A8 = exp(S - ln16): keeps exp(S) <= ~31 under the 448 fp8 max; the
    shift cancels exactly in the softmax normalization
  - colsum of A8 via DoubleRow ones-matmuls interleaved into the scores
    stream; the [2,128] ones lhsT replicates the sum across all 128 output
    partitions, so rb = 1/colsum needs just one DVE reciprocal, no broadcast
  - v8 = 16*v -> O' = psum_av * rb = 16*O (sigma ~0.8); proj psum = 256*proj
  - batch 0 evicts proj via one DVE op (psum/256 + x); batch 1 adds the
    residual on the PE itself (a 256*I fp32r matmul into the proj psum) so
    its eviction is a plain ACT identity — the tail then runs av-evictions
    (DVE) and output evictions (ACT) in parallel, by n-halves.

Engine split: PE does all matmuls + GN group-combine/broadcast indicator
matmuls; ACT evicts q/k (+bias) and exp(A) (+ batch-1 outputs); DVE runs
bn_stats, the packed GN chain, v/AV evictions and batch-0 residual adds;
Pool (GpSimd) takes part of the GN scale/shift (h8) applies.

GroupNorm stats feed from a bf16 copy of x loaded first (half the DMA bytes
of fp32; h8 is fp8 anyway so bf16 input precision is irrelevant) — the fp32
x needed for the exact residual loads later, off the critical path. GN
groups never span 128-channel tiles, so batch 0's chain runs per-tile,
releasing the first h8 tiles to the qkv matmuls several us earlier.

Infra notes: this walrus build allows ONE sync-wait per ISA instruction, so
_split_multi_waits() hoists extra waits onto same-engine NoOps. Pool
rejects accum_out ops and bn_stats (ISA engine check), and ldweights can't
be standalone for fp32 dtypes — paths chosen accordingly.
"""

import math

import numpy as np

B, C, N = 16, 512, 1024
G = 32
EPS = 1e-5
NCORES = 8
BPC = B // NCORES  # batches per core
CT = C // 128      # channel tiles (4)
NT = N // 128      # token tiles (8)
KC = C // 128      # contraction chunks over channels (4)
KM = N // 128      # contraction chunks over tokens (8)
HALF = 512
WS = 16.0          # host weight scale
SCALE = 1.0 / math.sqrt(C)
LN16 = 4.0 * math.log(2.0)

# packed fp32 small-constant tiles
# smallc [128, 16]: gnw(4) gnb(4) qkb16(8) — tiny, DMA'd first
SC_GNW = 0
SC_GNB = 4
SC_QKB = 8
SC_COLS = 16
# gind [128, GI_COLS]: GN indicator matmul operands + rank-1 bias operands
GI_GFWD = 0          # +32*t, width 32
GI_GBWD = 128        # +128*t, rows 0..31, width 128
GI_BEFFR = 640       # row0: 256*beff, 4 blocks of 128
GI_ONER = 1152       # row0: ones, 512 wide
GI_COLS = 1664

_CACHE = {}


def _build(with_beff):
    import concourse.bass as bass
    import concourse.tile as tile
    from concourse import mybir
    from contextlib import ExitStack

    f32 = mybir.dt.float32
    f8 = mybir.dt.float8e4
    Alu = mybir.AluOpType
    Act = mybir.ActivationFunctionType
    DR = mybir.MatmulPerfMode.DoubleRow

    nc = bass.Bass("TRN2", target_bir_lowering=False)

    def r(ap):
        return ap.bitcast(mybir.dt.float32r)

    x_d = nc.dram_tensor("x", [BPC, CT, 128, N], f32, kind="ExternalInput")
    x8_d = nc.dram_tensor("x8", [BPC, CT, 128, N], mybir.dt.bfloat16,
                          kind="ExternalInput")
    wqk_d = nc.dram_tensor("wqk", [128, KC, 2 * C], f8, kind="ExternalInput")
    wv_d = nc.dram_tensor("wv", [128, KC, C], f8, kind="ExternalInput")
    ow_d = nc.dram_tensor("ow", [128, KC, C], f8, kind="ExternalInput")
    smallc_d = nc.dram_tensor("smallc", [128, SC_COLS], f32, kind="ExternalInput")
    gind_d = nc.dram_tensor("gind", [128, GI_COLS], f32, kind="ExternalInput")
    id_d = nc.dram_tensor("id256", [128, 128], f32, kind="ExternalInput")
    out_d = nc.dram_tensor("out", [BPC, CT, 128, N], mybir.dt.bfloat16,
                           kind="ExternalOutput")

    with ExitStack() as ctx:
        ctx.enter_context(nc.allow_low_precision("fp8 DoubleRow path"))
        tc = ctx.enter_context(tile.TileContext(nc))
        consts = ctx.enter_context(tc.tile_pool(name="consts", bufs=1))
        xp = ctx.enter_context(tc.tile_pool(name="xp", bufs=2 * CT))
        xp8 = ctx.enter_context(tc.tile_pool(name="xp8", bufs=2 * CT))
        hp = ctx.enter_context(tc.tile_pool(name="hp", bufs=2))
        qp = ctx.enter_context(tc.tile_pool(name="qp", bufs=2))
        kp = ctx.enter_context(tc.tile_pool(name="kp", bufs=2))
        vp = ctx.enter_context(tc.tile_pool(name="vp", bufs=2))
        ap_ = ctx.enter_context(tc.tile_pool(name="ap_", bufs=2))
        op_ = ctx.enter_context(tc.tile_pool(name="op_", bufs=2))
        rp = ctx.enter_context(tc.tile_pool(name="rp", bufs=2))
        outp = ctx.enter_context(tc.tile_pool(name="outp", bufs=8))
        smallp = ctx.enter_context(tc.tile_pool(name="smallp", bufs=2))
        scrp = ctx.enter_context(tc.tile_pool(name="scrp", bufs=1))
        pmm = ctx.enter_context(tc.tile_pool(name="pmm", bufs=3, space="PSUM"))
        pcs = ctx.enter_context(tc.tile_pool(name="pcs", bufs=1, space="PSUM"))

        # --- const tiles (DMAs emitted inside the schedule below)
        smallc = consts.tile([128, SC_COLS], f32, tag="smallc", name="smallc")
        gind = consts.tile([128, GI_COLS], f32, tag="gind", name="gind")
        gnw4 = smallc[:, SC_GNW:SC_GNW + 4]
        gnb4 = smallc[:, SC_GNB:SC_GNB + 4]
        qkb = [smallc[:, SC_QKB + m:SC_QKB + m + 1] for m in range(2 * CT)]
        gfwd = [gind[:, GI_GFWD + G * t:GI_GFWD + G * (t + 1)] for t in range(CT)]
        gbwd = [gind[0:G, GI_GBWD + 128 * t:GI_GBWD + 128 * (t + 1)]
                for t in range(CT)]
        id256 = consts.tile([128, 128], f32, tag="id256", name="id256")
        wqk8 = consts.tile([128, KC, 2 * C], f8, tag="wqk8", name="wqk8")
        wv8 = consts.tile([128, KC, C], f8, tag="wv8", name="wv8")
        ow8 = consts.tile([128, KC, C], f8, tag="ow8", name="ow8")
        eps_t = consts.tile([G, 1], f32, tag="eps_t", name="eps_t")
        nc.vector.memset(eps_t, EPS)
        nl16 = consts.tile([128, 1], f32, tag="nl16", name="nl16")
        nc.vector.memset(nl16, -LN16)
        ones8 = consts.tile([128, 2, 128], f8, tag="ones8", name="ones8")
        nc.vector.memset(ones8, 1.0)
        scr = scrp.tile([128, N], f32, tag="scr", name="scr")  # Pool stats junk

        xt = {}
        x8t = {}
        ht = {}
        st8s = {}
        qt = {}
        kt = {}
        vt = {}
        at = {}
        ot = {}
        rbs = {}

        bf16 = mybir.dt.bfloat16

        def emit_x8loads(b, split_first=False):
            # bf16 copy of x: feeds GN stats + the h8 apply (h8 is fp8 anyway
            # so bf16 input precision is irrelevant); half the DMA bytes of
            # fp32 so the startup-critical stats path unblocks sooner
            x8t[b] = []
            for t in range(CT):
                x1 = xp8.tile([128, N], bf16, tag="x8", name=f"x8_{b}_{t}")
                if split_first and t == 0:
                    for h in range(2):
                        nc.sync.dma_start(
                            out=x1[:, h * HALF:(h + 1) * HALF],
                            in_=x8_d[b, t, :, h * HALF:(h + 1) * HALF])
                else:
                    nc.sync.dma_start(out=x1, in_=x8_d[b, t])
                x8t[b].append(x1)

        def emit_xloads(b, as_r=False):
            # fp32 x, only needed for the late residual-add; batch 1 is
            # consumed by an fp32r matmul so its DMA writes the r-typed view
            xt[b] = []
            for t in range(CT):
                x1 = xp.tile([128, N], f32, tag="x", name=f"x{b}_{t}")
                if as_r:
                    nc.sync.dma_start(out=r(x1), in_=r(x_d[b, t]))
                else:
                    nc.sync.dma_start(out=x1, in_=x_d[b, t])
                xt[b].append(x1)

        def emit_stats_dve(b, tiles=None):
            # bn_stats/bn_aggr -> st8 pairs (mean, var); per-tile var ->
            # E[x^2] fixup so it pipelines under the next tile's bn_stats
            tiles = tiles if tiles is not None else list(range(CT))
            if b not in st8s:
                st8s[b] = smallp.tile([128, 8], f32, tag="st8", name=f"st8_{b}")
            st8 = st8s[b]
            for t in tiles:
                st6 = smallp.tile([128, 2, 6], f32, tag=f"st6{t}", name=f"st6{b}_{t}")
                xv = x8t[b][t].rearrange("p (s f) -> p s f", s=2)
                for s_ in range(2):
                    nc.vector.bn_stats(out=st6[:, s_, :], in_=xv[:, s_, :])
                nc.vector.bn_aggr(out=st8[:, 2 * t:2 * t + 2], in_=st6)
                msq = smallp.tile([128, 1], f32, tag=f"msq{t}", name=f"msq{b}_{t}")
                nc.vector.tensor_mul(
                    out=msq, in0=st8[:, 2 * t:2 * t + 1], in1=st8[:, 2 * t:2 * t + 1])
                nc.vector.tensor_add(
                    out=st8[:, 2 * t + 1:2 * t + 2],
                    in0=st8[:, 2 * t + 1:2 * t + 2], in1=msq)

        def emit_stats_act(b, tiles):
            # raw sum / sum-sq per channel on ScalarE via accum_out — fills
            # the otherwise-idle ACT at startup for tile 0's critical path
            if b not in st8s:
                st8s[b] = smallp.tile([128, 8], f32, tag="st8", name=f"st8_{b}")
            st8 = st8s[b]
            for t in tiles:
                nc.scalar.activation(
                    out=scr, in_=x8t[b][t], func=Act.Identity,
                    accum_out=st8[:, 2 * t:2 * t + 1])
                nc.scalar.activation(
                    out=scr, in_=x8t[b][t], func=Act.Square,
                    accum_out=st8[:, 2 * t + 1:2 * t + 2])

        def emit_stats_pool(b):
            # raw sum / sum-sq per channel on the otherwise-idle Pool engine
            # via accum_out; the 1/N scaling folds into the group-combine
            # chain constants (st_scale)
            st8 = smallp.tile([128, 8], f32, tag="st8", name=f"st8_{b}")
            st8s[b] = st8
            for t in range(CT):
                nc.gpsimd.tensor_scalar(
                    out=scr, in0=x8t[b][t], scalar1=1.0, scalar2=0.0,
                    op0=Alu.mult, op1=Alu.add,
                    accum_out=st8[:, 2 * t:2 * t + 1],
                )
                nc.gpsimd.scalar_tensor_tensor(
                    out=scr, in0=x8t[b][t], scalar=1.0, in1=x8t[b][t],
                    op0=Alu.mult, op1=Alu.mult,
                    accum_out=st8[:, 2 * t + 1:2 * t + 2],
                )

        gn_state = {}

        def emit_gn_chain_act(b, st_scale, t):
            # single-tile chain variant fused onto ACT (which can read PSUM):
            # mean = gsum_e*s; b0 = eps - mean^2; sd = sqrt(gsum_o*s + b0);
            # only the reciprocal needs DVE. Back-to-back same-engine ops
            # avoid the cross-engine sem hops on the startup critical path.
            st8 = st8s[b]
            if b not in gn_state:
                gn_ps = pmm.tile([128, N], f32, tag="mm", name=f"gnps{b}")
                meanInv8 = smallp.tile([G, 8], f32, tag="mi8", name=f"mi8_{b}")
                gn_state[b] = (gn_ps, meanInv8)
            gn_ps, meanInv8 = gn_state[b]
            gsum8 = gn_ps[0:G, 0:8]
            nc.tensor.matmul(
                gsum8[:, 2 * t:2 * t + 2], gfwd[t], st8[:, 2 * t:2 * t + 2],
                start=True, stop=True, skip_group_check=True,
            )
            mcol = meanInv8[:, 2 * t:2 * t + 1]
            nc.scalar.activation(
                out=mcol, in_=gsum8[:, 2 * t:2 * t + 1], func=Act.Identity,
                scale=st_scale / 16.0)
            b0 = smallp.tile([G, 1], f32, tag=f"b0_{t}", name=f"b0_{b}_{t}")
            msq = smallp.tile([G, 1], f32, tag=f"am_{t}", name=f"am_{b}_{t}")
            nc.scalar.activation(out=msq, in_=mcol, func=Act.Square)
            nc.scalar.activation(out=b0, in_=msq, func=Act.Identity,
                                 scale=-1.0, bias=eps_t)
            sd = smallp.tile([G, 1], f32, tag=f"sd_{t}", name=f"sd_{b}_{t}")
            nc.scalar.activation(
                out=sd, in_=gsum8[:, 2 * t + 1:2 * t + 2], func=Act.Sqrt,
                scale=st_scale / 16.0, bias=b0)
            nc.vector.reciprocal(out=meanInv8[:, 2 * t + 1:2 * t + 2], in_=sd)
            return meanInv8

        def emit_gn_chain(b, st_scale, tiles=None):
            # group combine (PE indicator matmuls into a [G,8] psum slice) +
            # packed group-stat chain -> meanInv8 (mean at even, 1/sd at odd).
            # GN groups never span channel tiles, so the chain can run on a
            # tile subset (startup: release tiles 0/1 early).
            tiles = tiles if tiles is not None else list(range(CT))
            st8 = st8s[b]
            if b not in gn_state:
                gn_ps = pmm.tile([128, N], f32, tag="mm", name=f"gnps{b}")
                meanInv8 = smallp.tile([G, 8], f32, tag="mi8", name=f"mi8_{b}")
                gn_state[b] = (gn_ps, meanInv8)
            gn_ps, meanInv8 = gn_state[b]
            gsum8 = gn_ps[0:G, 0:8]
            for t in tiles:
                nc.tensor.matmul(
                    gsum8[:, 2 * t:2 * t + 2], gfwd[t], st8[:, 2 * t:2 * t + 2],
                    start=True, stop=True, skip_group_check=True,
                )
            t0, nt_ = tiles[0], len(tiles)
            gsv = gsum8.rearrange("p (t s) -> p t s", s=2)[:, t0:t0 + nt_, :]
            miv = meanInv8.rearrange("p (t s) -> p t s", s=2)[:, t0:t0 + nt_, :]
            nc.vector.tensor_scalar_mul(
                out=miv[:, :, 0:1], in0=gsv[:, :, 0:1], scalar1=st_scale / 16.0)
            msq2 = smallp.tile([G, nt_], f32, tag=f"msq2_{t0}",
                               name=f"msq2_{b}_{t0}")
            nc.vector.tensor_mul(out=msq2, in0=miv[:, :, 0:1], in1=miv[:, :, 0:1])
            var4 = smallp.tile([G, nt_], f32, tag=f"var4_{t0}",
                               name=f"var4_{b}_{t0}")
            nc.vector.scalar_tensor_tensor(
                out=var4, in0=gsv[:, :, 1:2], scalar=st_scale / 16.0, in1=msq2,
                op0=Alu.mult, op1=Alu.subtract,
            )
            sd4 = smallp.tile([G, nt_], f32, tag=f"sd4_{t0}",
                              name=f"sd4_{b}_{t0}")
            nc.scalar.activation(out=sd4, in_=var4, func=Act.Sqrt, bias=eps_t)
            nc.vector.reciprocal(out=miv[:, :, 1:2], in_=sd4)
            return meanInv8

        apply_state = {}

        def emit_gn_apply(b, meanInv8, h_engines, tiles=None):
            # broadcast group mean/inv-sd to channels (PE), then per-channel
            # h8 = x*a1 + t1n on the engines named in h_engines
            tiles = tiles if tiles is not None else list(range(CT))
            if b not in apply_state:
                mc_ps = pmm.tile([128, N], f32, tag="mm", name=f"mcps{b}")
                a1_4 = smallp.tile([128, 4], f32, tag="a14", name=f"a14_{b}")
                t1n4 = smallp.tile([128, 4], f32, tag="t1n4", name=f"t1n4_{b}")
                h8 = hp.tile([128, KC, N], f8, tag="h8", name=f"h8_{b}")
                apply_state[b] = (mc_ps, a1_4, t1n4, h8)
            mc_ps, a1_4, t1n4, h8 = apply_state[b]
            mc8 = mc_ps[:, 0:8]
            for t in tiles:
                nc.tensor.matmul(
                    mc8[:, 2 * t:2 * t + 2], gbwd[t], meanInv8[:, 2 * t:2 * t + 2],
                    start=True, stop=True, skip_group_check=True,
                )
            t0, nt_ = tiles[0], len(tiles)
            mcv = mc8.rearrange("p (t s) -> p t s", s=2)[:, t0:t0 + nt_, :]
            a1s = a1_4[:, t0:t0 + nt_]
            t1s = t1n4[:, t0:t0 + nt_]
            nc.vector.tensor_mul(out=a1s, in0=mcv[:, :, 1:2],
                                 in1=gnw4[:, t0:t0 + nt_])
            tmp4 = smallp.tile([128, nt_], f32, tag=f"tmp4_{t0}",
                               name=f"tmp4_{b}_{t0}")
            nc.vector.tensor_mul(out=tmp4, in0=mcv[:, :, 0:1], in1=a1s)
            nc.vector.tensor_sub(out=t1s, in0=gnb4[:, t0:t0 + nt_], in1=tmp4)
            for t in tiles:
                eng = h_engines[t]
                if eng == "act":
                    nc.scalar.activation(
                        out=h8[:, t, :], in_=x8t[b][t], func=Act.Identity,
                        scale=a1_4[:, t:t + 1], bias=t1n4[:, t:t + 1],
                    )
                else:
                    e = nc.vector if eng == "dve" else nc.gpsimd
                    e.tensor_scalar(
                        out=h8[:, t, :], in0=x8t[b][t],
                        scalar1=a1_4[:, t:t + 1], scalar2=t1n4[:, t:t + 1],
                        op0=Alu.mult, op1=Alu.add,
                    )
            ht[b] = h8

        def emit_qkv(b, v_eng, qk_engs=None, qk_first=False):
            # interleave ACT-drained (q/k) and v psum groups so both
            # eviction engines stay busy; qk_engs overrides the qk eviction
            # engine per half ("act" or ("dve","pool") pairs)
            h8 = ht[b]
            q8 = qp.tile([128, KC, N], f8, tag="q8", name=f"q8_{b}")
            k8 = kp.tile([128, KC, N], f8, tag="k8", name=f"k8_{b}")
            v8 = vp.tile([128, NT, HALF], f8, tag="v8", name=f"v8_{b}")
            qt[b], kt[b], vt[b] = q8, k8, v8

            def qk_group(mt):
                ps = pmm.tile([128, N], f32, tag="mm", name=f"psqk{b}_{mt}")
                for c in range(0, KC, 2):
                    for h in range(2):
                        nc.tensor.matmul(
                            ps[:, h * HALF:(h + 1) * HALF],
                            wqk8[:, c:c + 2, mt * 128:(mt + 1) * 128],
                            h8[:, c:c + 2, h * HALF:(h + 1) * HALF],
                            start=(c == 0), stop=(c == KC - 2),
                            perf_mode=DR, skip_group_check=True,
                        )
                dest = q8 if mt < CT else k8
                eng = qk_engs[mt] if qk_engs else "act"
                if eng == "act":
                    nc.scalar.activation(
                        out=dest[:, mt % CT, :], in_=ps, func=Act.Identity,
                        bias=qkb[mt],
                    )
                elif eng == "dve":
                    nc.vector.tensor_scalar(
                        out=dest[:, mt % CT, :], in0=ps,
                        scalar1=1.0, scalar2=qkb[mt],
                        op0=Alu.mult, op1=Alu.add,
                    )
                else:
                    for h in range(2):
                        e = nc.vector if eng[h] == "dve" else nc.gpsimd
                        e.tensor_scalar(
                            out=dest[:, mt % CT, h * HALF:(h + 1) * HALF],
                            in0=ps[:, h * HALF:(h + 1) * HALF],
                            scalar1=1.0, scalar2=qkb[mt],
                            op0=Alu.mult, op1=Alu.add,
                        )

            def v_group(nt):
                ps = pmm.tile([128, N], f32, tag="mm", name=f"psv{b}_{nt}")
                for j in range(2):
                    for c in range(0, KC, 2):
                        nc.tensor.matmul(
                            ps[:, j * HALF:(j + 1) * HALF],
                            h8[:, c:c + 2, (nt + j) * 128:(nt + j + 1) * 128],
                            wv8[:, c:c + 2, :],
                            start=(c == 0), stop=(c == KC - 2),
                            perf_mode=DR, skip_group_check=True,
                        )
                if v_eng == "act":
                    nc.scalar.copy(out=v8[:, nt:nt + 2, :], in_=ps)
                elif v_eng == "pool":
                    nc.gpsimd.tensor_copy(out=v8[:, nt:nt + 2, :], in_=ps)
                elif v_eng == "dve+pool":
                    nc.vector.tensor_copy(out=v8[:, nt, :], in_=ps[:, 0:HALF])
                    nc.gpsimd.tensor_copy(out=v8[:, nt + 1, :],
                                          in_=ps[:, HALF:N])
                else:
                    nc.vector.tensor_copy(out=v8[:, nt:nt + 2, :], in_=ps)

            if qk_first:
                for g in range(4):
                    qk_group(g)
                    qk_group(4 + g)
                for g in range(4):
                    v_group(2 * g)
            else:
                for g in range(4):
                    qk_group(g)
                    v_group(2 * g)
                    qk_group(4 + g)

        css = {}
        avheld = {}

        def emit_scores(b, mts, hold_av0=False):
            # scores + exp eviction; after each odd mt, the colsum chunk for
            # (mt-1, mt) is accumulated into the held cs psum via a DoubleRow
            # ones-matmul (its [2,128] ones lhsT replicates the sum across
            # all 128 output partitions, so rb needs no separate broadcast).
            # With hold_av0, the attn@V chunk for output channels 0..127 also
            # accumulates early into a held psum, shortening the tail.
            q8, k8 = qt[b], kt[b]
            if b not in at:
                at[b] = ap_.tile([128, KM, N], f8, tag="a8", name=f"a8_{b}")
                css[b] = pcs.tile([128, N], f32, tag="cs", name=f"cs{b}")
            a8 = at[b]
            cs = css[b]
            for mt in mts:
                ps = pmm.tile([128, N], f32, tag="mm", name=f"pss{b}_{mt}")
                for c in range(0, KC, 2):
                    for h in range(2):
                        nc.tensor.matmul(
                            ps[:, h * HALF:(h + 1) * HALF],
                            k8[:, c:c + 2, mt * 128:(mt + 1) * 128],
                            q8[:, c:c + 2, h * HALF:(h + 1) * HALF],
                            start=(c == 0), stop=(c == KC - 2),
                            perf_mode=DR, skip_group_check=True,
                        )
                nc.scalar.activation(
                    out=a8[:, mt, :], in_=ps, func=Act.Exp,
                    scale=SCALE / (WS * WS), bias=nl16,
                )
                if mt % 2 == 1:
                    m = mt - 1
                    for h in range(2):
                        nc.tensor.matmul(
                            cs[:, h * HALF:(h + 1) * HALF],
                            ones8[:, :, :],
                            a8[:, m:m + 2, h * HALF:(h + 1) * HALF],
                            start=(m == 0), stop=(m == KM - 2),
                            perf_mode=DR, skip_group_check=True,
                        )
                    if b in avheld:
                        for h in range(2):
                            nc.tensor.matmul(
                                avheld[b][:, h * HALF:(h + 1) * HALF],
                                vt[b][:, m:m + 2, 0:128],
                                a8[:, m:m + 2, h * HALF:(h + 1) * HALF],
                                start=(m == 0), stop=(m == KM - 2),
                                perf_mode=DR, skip_group_check=True,
                            )

        def emit_rb(b):
            rb = rp.tile([128, N], f32, tag="rb", name=f"rb{b}")
            for h in range(2):
                nc.vector.reciprocal(
                    out=rb[:, h * HALF:(h + 1) * HALF],
                    in_=css[b][:, h * HALF:(h + 1) * HALF])
            rbs[b] = rb

        def emit_av(b, cts, half_evict=False):
            # half_evict: evict all four h0 halves first so the proj h0
            # matmuls can start ~2.5us earlier in the tail
            a8, v8 = at[b], vt[b]
            if b not in ot:
                ot[b] = op_.tile([128, KC, N], f8, tag="o8", name=f"o8_{b}")
            o8 = ot[b]
            pss = {}
            for ct_ in cts:
                ps = pmm.tile([128, N], f32, tag="mm", name=f"pso{b}_{ct_}")
                pss[ct_] = ps
                for m in range(0, KM, 2):
                    for h in range(2):
                        nc.tensor.matmul(
                            ps[:, h * HALF:(h + 1) * HALF],
                            v8[:, m:m + 2, ct_ * 128:(ct_ + 1) * 128],
                            a8[:, m:m + 2, h * HALF:(h + 1) * HALF],
                            start=(m == 0), stop=(m == KM - 2),
                            perf_mode=DR, skip_group_check=True,
                        )
                if half_evict:
                    nc.vector.tensor_mul(
                        out=o8[:, ct_, 0:HALF], in0=ps[:, 0:HALF],
                        in1=rbs[b][:, 0:HALF])
                else:
                    nc.vector.tensor_mul(out=o8[:, ct_, :], in0=ps, in1=rbs[b])
            if half_evict:
                for ct_ in cts:
                    nc.vector.tensor_mul(
                        out=o8[:, ct_, HALF:N], in0=pss[ct_][:, HALF:N],
                        in1=rbs[b][:, HALF:N])

        def emit_proj(b, ts_, res_pe=False, stt_eng="dve"):
            # res_pe: add the residual on the PE via a 256*I fp32r matmul so
            # the eviction is a plain ACT identity (frees DVE in the tail)
            o8 = ot[b]
            for t in ts_:
                ps = pmm.tile([128, N], f32, tag="mm", name=f"psp{b}_{t}")
                for c in range(0, KC, 2):
                    for h in range(2):
                        nc.tensor.matmul(
                            ps[:, h * HALF:(h + 1) * HALF],
                            ow8[:, c:c + 2, t * 128:(t + 1) * 128],
                            o8[:, c:c + 2, h * HALF:(h + 1) * HALF],
                            start=(c == 0),
                            stop=(c == KC - 2) and not with_beff and not res_pe,
                            perf_mode=DR, skip_group_check=True,
                        )
                if with_beff:
                    # rank-1 bias add: psum += (256*beff) x ones_row
                    for h in range(2):
                        nc.tensor.matmul(
                            ps[:, h * HALF:(h + 1) * HALF],
                            gind[0:1, GI_BEFFR + 128 * t:GI_BEFFR + 128 * (t + 1)],
                            gind[0:1, GI_ONER:GI_ONER + HALF],
                            start=False, stop=(h == 1) and not res_pe,
                            skip_group_check=True,
                        )
                f1 = outp.tile([128, N], bf16, tag="f", name=f"f{b}_{t}")
                if res_pe:
                    for h in range(2):
                        nc.tensor.matmul(
                            ps[:, h * HALF:(h + 1) * HALF],
                            r(id256),
                            r(xt[b][t][:, h * HALF:(h + 1) * HALF]),
                            start=False, stop=(h == 1), skip_group_check=True,
                        )
                    nc.scalar.activation(
                        out=f1, in_=ps, func=Act.Identity,
                        scale=1.0 / (WS * WS))
                else:
                    e = nc.vector if stt_eng == "dve" else nc.gpsimd
                    e.scalar_tensor_tensor(
                        out=f1, in0=ps, scalar=1.0 / (WS * WS), in1=xt[b][t],
                        op0=Alu.mult, op1=Alu.add,
                    )
                for h in range(2):
                    nc.sync.dma_start(
                        out=out_d[b, t, :, h * HALF:(h + 1) * HALF],
                        in_=f1[:, h * HALF:(h + 1) * HALF])

        def emit_proj_halved(b):
            # tail variant: per n-half proj with PE residual-add and ACT
            # eviction, so stores start while the h1 halves still compute
            o8 = ot[b]
            f1s = {}
            for t in range(CT):
                f1s[t] = outp.tile([128, N], bf16, tag="f", name=f"f{b}_{t}")
            for h in range(2):
                pss = {}
                for t in range(CT):
                    ps = pmm.tile([128, HALF], f32, tag="mm",
                                  name=f"psp{b}_{t}_{h}")
                    pss[t] = ps
                    for c in range(0, KC, 2):
                        nc.tensor.matmul(
                            ps, ow8[:, c:c + 2, t * 128:(t + 1) * 128],
                            o8[:, c:c + 2, h * HALF:(h + 1) * HALF],
                            start=(c == 0), stop=False,
                            perf_mode=DR, skip_group_check=True,
                        )
                    if with_beff:
                        nc.tensor.matmul(
                            ps,
                            gind[0:1, GI_BEFFR + 128 * t:GI_BEFFR + 128 * (t + 1)],
                            gind[0:1, GI_ONER:GI_ONER + HALF],
                            start=False, stop=False, skip_group_check=True,
                        )
                    nc.tensor.matmul(
                        ps, r(id256),
                        r(xt[b][t][:, h * HALF:(h + 1) * HALF]),
                        start=False, stop=True, skip_group_check=True,
                    )
                for t in range(CT):
                    fh = f1s[t][:, h * HALF:(h + 1) * HALF]
                    nc.scalar.activation(
                        out=fh, in_=pss[t], func=Act.Identity,
                        scale=1.0 / (WS * WS))
                    nc.sync.dma_start(
                        out=out_d[b, t, :, h * HALF:(h + 1) * HALF], in_=fh)

        # --- software-pipelined emission across the two batches
        emit_x8loads(0)
        nc.sync.dma_start(out=smallc, in_=smallc_d[:, :])
        # gfwd block gates the first GN matmul — land it first, rest later
        gi_cols = GI_COLS if with_beff else GI_BEFFR
        nc.sync.dma_start(out=gind[:, 0:GI_GBWD], in_=gind_d[:, 0:GI_GBWD])
        nc.sync.dma_start(out=gind[:, GI_GBWD:gi_cols],
                          in_=gind_d[:, GI_GBWD:gi_cols])
        nc.sync.dma_start(out=wqk8, in_=wqk_d[:, :, :])
        nc.sync.dma_start(out=wv8, in_=wv_d[:, :, :])
        emit_stats_act(0, [0])
        mi0 = emit_gn_chain_act(0, 1.0 / N, 0)
        emit_gn_apply(0, mi0, ["act", "dve", "pool", "act"], [0])
        emit_stats_dve(0, [1])
        emit_gn_chain_act(0, 1.0, 1)
        emit_gn_apply(0, mi0, ["act", "dve", "pool", "act"], [1])
        emit_stats_dve(0, [2, 3])
        emit_gn_chain(0, 1.0, [2, 3])
        emit_gn_apply(0, mi0, ["act", "dve", "pool", "act"], [2, 3])
        emit_x8loads(1)
        emit_qkv(0, v_eng="dve")
        emit_xloads(0, as_r=True)
        nc.sync.dma_start(out=ow8, in_=ow_d[:, :, :])
        nc.sync.dma_start(out=r(id256), in_=r(id_d[:, :]))
        emit_stats_dve(1)
        emit_scores(0, [0, 1])
        mi1 = emit_gn_chain(1, 1.0)   # sqrt slots in after A(0,0..1)
        emit_scores(0, [2, 3])
        emit_gn_apply(1, mi1, ["dve", "act", "pool", "pool"])
        emit_scores(0, [4, 5, 6, 7])
        emit_qkv(1, v_eng="dve")     # PE filler while ACT drains A(0)
        emit_xloads(1, as_r=True)
        emit_rb(0)
        # interleave av(0) (DVE-drained) with scores(1) (ACT-drained) and
        # proj(0) so both eviction engines stay fed
        emit_av(0, [0])
        emit_scores(1, [0, 1])
        emit_av(0, [1])
        emit_scores(1, [2, 3])
        emit_av(0, [2])
        emit_scores(1, [4, 5])
        emit_av(0, [3])
        emit_scores(1, [6, 7])
        emit_rb(1)
        emit_proj(0, [0, 1, 2, 3], res_pe=True)
        emit_av(1, [0, 1, 2, 3], half_evict=True)
        emit_proj_halved(1)

    _split_multi_waits(nc)
    return nc


def _split_multi_waits(nc):
    """This neuronxcc walrus supports one sync-wait per ISA instruction.

    Tile emits instructions with several waits; hoist all but the last onto
    same-engine NoOps inserted immediately before (engine sequencers execute
    waits in order, so this is semantically identical).
    """
    from concourse import mybir

    n = 0
    for f in nc.m.functions:
        for bb in f.blocks:
            insts = bb.instructions
            out = []
            for inst in insts:
                si = inst.sync_info
                if si is not None and si.on_wait and len(si.on_wait) > 1:
                    waits = list(si.on_wait)
                    for w in waits[:-1]:
                        nop = mybir.InstNoOp(name=f"WSPLIT-{n}", ins=[], outs=[])
                        n += 1
                        nop.engine = inst.engine
                        nop.sync_info = mybir.SyncInfo(on_wait=[w], on_update=[])
                        out.append(nop)
                    inst.sync_info = mybir.SyncInfo(
                        on_wait=[waits[-1]], on_update=list(si.on_update or [])
                    )
                out.append(inst)
            if n:
                bb.instructions = out
    return nc


def _prep_consts(qkv_w, qkv_b, out_w, out_b, gn_w, gn_b):
    from ml_dtypes import float8_e4m3fn

    f = np.float32
    # lhsT chunk layouts: [p, t, o] = W.T[128t+p, o], scaled x16, fp8
    wqk = np.ascontiguousarray(
        (WS * qkv_w[:2 * C]).T.reshape(KC, 128, 2 * C).transpose(1, 0, 2)
    ).astype(float8_e4m3fn)
    wv = np.ascontiguousarray(
        (WS * qkv_w[2 * C:]).T.reshape(KC, 128, C).transpose(1, 0, 2)
    ).astype(float8_e4m3fn)
    ow = np.ascontiguousarray(
        (WS * out_w).T.reshape(KC, 128, C).transpose(1, 0, 2)
    ).astype(float8_e4m3fn)
    smallc = np.zeros((128, SC_COLS), dtype=f)
    for t in range(CT):
        cs = slice(128 * t, 128 * (t + 1))
        smallc[:, SC_GNW + t] = gn_w[cs]
        smallc[:, SC_GNB + t] = gn_b[cs]
    for mt in range(2 * CT):
        smallc[:, SC_QKB + mt] = WS * qkv_b[128 * mt:128 * (mt + 1)]
    beff = out_w @ qkv_b[2 * C:] + out_b
    with_beff = bool(np.any(beff != 0.0))
    gind = np.zeros((128, GI_COLS), dtype=f)
    for t in range(CT):
        for p_ in range(128):
            gind[p_, GI_GFWD + G * t + (128 * t + p_) // 16] = 1.0
            gind[(128 * t + p_) // 16, GI_GBWD + 128 * t + p_] = 1.0
    gind[0, GI_BEFFR:GI_BEFFR + C] = (WS * WS) * beff
    gind[0, GI_ONER:GI_ONER + HALF] = 1.0
    id256 = np.zeros((128, 128), dtype=f)
    id256[np.arange(128), np.arange(128)] = WS * WS
    return dict(wqk=wqk, wv=wv, ow=ow, smallc=smallc, gind=gind,
                id256=id256), with_beff


def kernel(x, gn_w, gn_b, qkv_w, qkv_b, out_w, out_b):
    from concourse.bass_utils import run_bass_kernel_spmd

    x = np.ascontiguousarray(np.asarray(x, dtype=np.float32))
    consts, with_beff = _prep_consts(
        np.asarray(qkv_w, np.float32), np.asarray(qkv_b, np.float32),
        np.asarray(out_w, np.float32), np.asarray(out_b, np.float32),
        np.asarray(gn_w, np.float32), np.asarray(gn_b, np.float32),
    )
    from ml_dtypes import bfloat16
    xr = x.reshape(NCORES, BPC, CT, 128, N)
    x8r = xr.astype(bfloat16)
    in_maps = [dict(x=np.ascontiguousarray(xr[i]), x8=np.ascontiguousarray(x8r[i]),
                    **consts) for i in range(NCORES)]

    key = ("nc", with_beff)
    if key not in _CACHE:
        _CACHE[key] = _build(with_beff)
    _CACHE["nc"] = _CACHE[key]
    res = run_bass_kernel_spmd(
        _CACHE[key], in_maps, core_ids=list(range(NCORES)),
        trace=_CACHE.get("trace", False),
    )
    _CACHE["last"] = res
    out = np.stack([np.asarray(r["out"], dtype=np.float32)
                    for r in res.results])  # [8, BPC, CT, 128, N]
    return out.reshape(B, C, 32, 32)

